# revision 1
# baseline (speedup 1.0000x reference)
"""Distributed GQA causal attention forward on 8 TRN2 NeuronCores.

Problem shapes: residual [B=2, S=2048, D=2048]; W_Q/W_O [32, 64, 2048];
W_K/W_V [8, 64, 2048]; GQA rep=4; causal softmax attention; out [2, 2048, 2048].

Sharding (tensor parallel over heads, following the GQA structure):
  core c owns q-heads [4c, 4c+4) and kv-head c — exactly one GQA group, so
  attention is fully local. Each core computes Q/K/V projections for its
  heads over the full sequence, flash-style causal attention, and a partial
  output projection; partial outputs are summed with a chunked bf16
  ReduceScatter so each core emits a disjoint 512-row shard that the host
  reassembles.

All matmul operands are bf16 (fp32 PSUM accumulation); the scores scale
1/sqrt(64) is folded into W_Q on the host. Softmax skips max-subtraction
(logits are bounded ~|5| for this data distribution) and row-sums come from a
ones-column appended to V. Scores matmuls contract over d_head=64, so head
pairs are packed into PE row groups (0-63 / 64-127) to run concurrently;
K^T is stored duplicated across both partition halves to satisfy the
matmul base-partition constraint.
"""

import sys

for _p in ("/opt/trn_rl_repo", "/root/.axon_site/_ro/trn_rl_repo"):
    if _p not in sys.path:
        sys.path.insert(0, _p)

import numpy as np
from concourse import bacc, mybir, tile
from concourse import bass_utils

N_CORES = 8
B, S, D = 2, 2048, 2048
NH, NKV, DH = 32, 8, 64
NH_LOC = NH // N_CORES  # 4 q-heads per core
SEQ = B * S  # 4096 global rows, b-major
NHL = NH_LOC * DH  # 256 local q-head dim
P = 128
QG = 512  # q-group size (4 tiles of 128)
N_RCHUNK = SEQ // QG  # 8
N_DT = D // P  # 16 d-tiles
N_KT = S // P  # 16 key blocks per batch
N_CHUNK = 8  # ReduceScatter chunks: one per q-group (512 rows)

BF16 = mybir.dt.bfloat16
F32 = mybir.dt.float32
NP_BF16 = mybir.dt.np(BF16)

_compiled = None


def _build():
    nc = bacc.Bacc("TRN2", target_bir_lowering=False, debug=False, num_devices=N_CORES)

    resid_t = nc.dram_tensor("resid_t", [D, SEQ], BF16, kind="ExternalInput")
    wqt = nc.dram_tensor("wqt", [D, NHL], BF16, kind="ExternalInput")
    wkvt = nc.dram_tensor("wkvt", [D, 2 * DH], BF16, kind="ExternalInput")
    wo = nc.dram_tensor("wo", [NHL, D], BF16, kind="ExternalInput")
    mask = nc.dram_tensor("mask", [P, P], BF16, kind="ExternalInput")
    ident = nc.dram_tensor("ident", [P, P], F32, kind="ExternalInput")
    out = nc.dram_tensor("out", [SEQ // N_CORES, D], F32, kind="ExternalOutput")

    rs_in = [
        nc.dram_tensor(f"rs_in{k}", [SEQ // N_CHUNK, D], BF16, kind="Internal")
        for k in range(N_CHUNK)
    ]
    rs_out = [
        nc.dram_tensor(f"rs_out{k}", [QG // N_CORES, D], BF16, kind="Internal")
        for k in range(N_CHUNK)
    ]
    rg = [list(range(N_CORES))]
    COPY = mybir.ActivationFunctionType.Copy
    EXP = mybir.ActivationFunctionType.Exp

    with tile.TileContext(nc) as tc:
        with (
            tc.tile_pool(name="persist", bufs=1) as pp,
            tc.tile_pool(name="stream", bufs=3) as sp,
            tc.tile_pool(name="rstream", bufs=8) as rp,
            tc.tile_pool(name="pstream", bufs=4) as xp,
            tc.tile_pool(name="outbuf", bufs=3) as op,
        ):
            # ---- persistent SBUF tensors ----
            qT_sb = [pp.tile([P, SEQ], BF16, name=f"qT{i}") for i in range(2)]
            kT_sb = pp.tile([P, SEQ], BF16, name="kT")  # K^T duplicated in both halves
            v_sb = [pp.tile([P, P], BF16, name=f"v{rt}") for rt in range(SEQ // P)]
            attn_sb = [pp.tile([P, SEQ], BF16, name=f"attn{i}") for i in range(2)]
            wqt_sb = [pp.tile([P, NHL], BF16, name=f"wqt{i}") for i in range(N_DT)]
            wkvt_sb = [pp.tile([P, 2 * DH], BF16, name=f"wkvt{i}") for i in range(N_DT)]
            wo_sb = [pp.tile([P, D], BF16, name=f"wo{i}") for i in range(2)]
            mask_sb = pp.tile([P, P], BF16, name="mask")
            ident_sb = pp.tile([P, P], F32, name="ident")

            nc.sync.dma_start(mask_sb[:], mask.ap())
            nc.sync.dma_start(ident_sb[:], ident.ap())
            for i in range(N_DT):
                nc.sync.dma_start(wqt_sb[i][:], wqt.ap()[i * P : (i + 1) * P, :])
                nc.sync.dma_start(wkvt_sb[i][:], wkvt.ap()[i * P : (i + 1) * P, :])
            for i in range(2):
                nc.sync.dma_start(wo_sb[i][:], wo.ap()[i * P : (i + 1) * P, :])
            for rt in range(SEQ // P):
                nc.vector.memset(v_sb[rt][:, 0:DH], 0.0)
                nc.vector.memset(v_sb[rt][:, 0:1], 1.0)

            # ---- phase A: Q / K / V projections ----
            # residual^T streamed in [128 d, 512 row] tiles via DMA transpose;
            # Q^T accumulated in [128 nh, 512] psum, K^T/V^T in a shared
            # [128, 512] psum (rows 0:64 = K^T, 64:128 = V^T).
            with tc.tile_pool(name="psA", bufs=2, space="PSUM") as psA:
                for rc in range(N_RCHUNK):
                    r0 = rc * QG
                    qp = [psA.tile([P, QG], F32, tag=f"qp{i}", name=f"qp{i}") for i in range(2)]
                    kvp = psA.tile([P, QG], F32, tag="kvp", name="kvp")
                    for dt_ in range(N_DT):
                        rt_tile = rp.tile([P, QG], BF16, tag="residT", name="residT")
                        nc.sync.dma_start(
                            rt_tile[:],
                            resid_t.ap()[dt_ * P : (dt_ + 1) * P, r0 : r0 + QG],
                        )
                        st = dict(start=(dt_ == 0), stop=(dt_ == N_DT - 1))
                        for hb in range(2):
                            nc.tensor.matmul(
                                qp[hb][:],
                                wqt_sb[dt_][:, hb * P : (hb + 1) * P],
                                rt_tile[:],
                                **st,
                            )
                        nc.tensor.matmul(kvp[:], wkvt_sb[dt_][:], rt_tile[:], **st)
                    for hb in range(2):
                        nc.scalar.activation(qT_sb[hb][:, r0 : r0 + QG], qp[hb][:], COPY)
                    nc.scalar.activation(kT_sb[0:DH, r0 : r0 + QG], kvp[0:DH, :], COPY)
                    nc.vector.tensor_copy(kT_sb[DH : 2 * DH, r0 : r0 + QG], kvp[0:DH, :])
                    # V^T -> V via PE transpose (per 128-key tile)
                    vt_tmp = sp.tile([DH, QG], F32, tag="vt_tmp", name="vt_tmp")
                    nc.vector.tensor_copy(vt_tmp[:], kvp[DH : 2 * DH, :])
                    for j in range(QG // P):
                        vtr = psA.tile([P, DH], F32, tag="vtr", name="vtr")
                        nc.tensor.transpose(
                            vtr[:], vt_tmp[:, j * P : (j + 1) * P], ident_sb[0:DH, 0:DH]
                        )
                        nc.vector.tensor_copy(v_sb[rc * 4 + j][:, DH : 2 * DH], vtr[:])

            # ---- phases B+C interleaved per ReduceScatter chunk ----
            # chunk kk = q-group g = kk % 4 of batch kk // 4 (512 q rows).
            # Head pairs (2i, 2i+1) run in PE row groups 0/64, interleaved per
            # key block so the PE fills each pair's exp-wait with the other
            # pair's MMs. O-projection PSUM drains via the ACT engine (idle at
            # group boundaries) so shared score slots recycle fast.
            with (
                tc.tile_pool(name="psS", bufs=2, space="PSUM") as psS,
                tc.tile_pool(name="psT", bufs=4, space="PSUM") as psT,
            ):
                last_osb_dma = None

                def emit_oproj(kc):
                    nonlocal last_osb_dma
                    for qt in range(4):
                        col0 = (kc * 4 + qt) * P
                        o_sb = op.tile([P, D], BF16, tag="o_sb", name="o_sb")
                        for dti in range(4):
                            ops = psS.tile([P, 2, QG], F32, tag="sc", name="sc")
                            for hb in range(2):
                                nc.tensor.matmul(
                                    ops[:, 0, :],
                                    attn_sb[hb][:, col0 : col0 + P],
                                    wo_sb[hb][:, dti * 512 : (dti + 1) * 512],
                                    start=(hb == 0),
                                    stop=(hb == 1),
                                )
                            nc.scalar.activation(
                                o_sb[:, dti * 512 : (dti + 1) * 512], ops[:, 0, :], COPY
                            )
                        last_osb_dma = nc.sync.dma_start(
                            rs_in[kc].ap()[qt * P : (qt + 1) * P, :], o_sb[:]
                        )
                    nc.gpsimd.collective_compute(
                        "ReduceScatter",
                        mybir.AluOpType.add,
                        replica_groups=rg,
                        ins=[rs_in[kc].ap().opt()],
                        outs=[rs_out[kc].ap().opt()],
                    )

                for kk in range(N_CHUNK):
                    b, g = kk // 4, kk % 4
                    at = [
                        psT.tile([P, QG], F32, tag="at", name="at")
                        for _ in range(4)
                    ]
                    for kb in range(4 * g + 4):
                        j = max(0, kb - 4 * g)
                        qoff = b * S + g * QG + j * P
                        n = QG - j * P
                        k0 = b * S + kb * P
                        pts = []
                        for hb in range(2):
                            sc = psS.tile([P, 2, QG], F32, tag="sc", name="sc")
                            for u in range(2):
                                lo = u * DH
                                nc.tensor.matmul(
                                    sc[:, u, :n],
                                    kT_sb[lo : lo + DH, k0 : k0 + P],
                                    qT_sb[hb][lo : lo + DH, qoff : qoff + n],
                                    start=True,
                                    stop=True,
                                )
                            pt = xp.tile([P, 2, QG], BF16, tag="p_sb", name="p_sb")
                            nc.scalar.activation(pt[:, :, :n], sc[:, :, :n], EXP)
                            if kb >= 4 * g:
                                nc.vector.tensor_tensor(
                                    pt[:, :, 0:P],
                                    pt[:, :, 0:P],
                                    mask_sb[:].unsqueeze(1).broadcast_to([P, 2, P]),
                                    mybir.AluOpType.mult,
                                )
                            pts.append(pt)
                        for hb in range(2):
                            for u in range(2):
                                nc.tensor.matmul(
                                    at[2 * hb + u][:, j * P : QG],
                                    v_sb[b * N_KT + kb][:],
                                    pts[hb][:, u, :n],
                                    start=(kb == 0),
                                    stop=(kb == 4 * g + 3),
                                )
                    recips = []
                    for hb in range(2):
                        for u in range(2):
                            a = at[2 * hb + u]
                            sm = sp.tile([1, QG], F32, tag="sm", name="sm")
                            nc.vector.tensor_copy(sm[:], a[0:1, :])
                            recip = sp.tile([1, QG], F32, tag="recip", name="recip")
                            nc.vector.reciprocal_approx_fast(recip[:], sm[:])
                            recips.append(recip)
                    bcs = []
                    for i in range(4):
                        bc = sp.tile([DH, QG], F32, tag="bc", name="bc")
                        nc.gpsimd.partition_broadcast(bc[:], recips[i][:])
                        bcs.append(bc)
                    for hb in range(2):
                        for u in range(2):
                            hp = u * DH
                            nc.vector.tensor_tensor(
                                attn_sb[hb][
                                    hp : hp + DH, b * S + g * QG : b * S + (g + 1) * QG
                                ],
                                at[2 * hb + u][DH : 2 * DH, :],
                                bcs[2 * hb + u][:],
                                mybir.AluOpType.mult,
                            )
                    if kk > 0:
                        emit_oproj(kk - 1)
                if True:
                    emit_oproj(N_CHUNK - 1)

                # ---- readback: bf16 shard -> f32 output ----
                # explicitly ordered after the last chunk's output DMA so the
                # collective-wait doesn't head-of-line block the Sync/DVE queues
                # mid-kernel (engine queues are in-order).
                from concourse.tile_rust import add_dep_helper

                RB = QG // N_CORES  # 64 rows per chunk shard
                for kk in range(N_CHUNK):
                    rb = op.tile([RB, D], BF16, tag="rb", name="rb")
                    rb_dma = nc.sync.dma_start(rb[:], rs_out[kk].ap())
                    add_dep_helper(
                        rb_dma.ins, last_osb_dma.ins, False, "readback after compute"
                    )
                    rb32 = op.tile([RB, D], F32, tag="rb32", name="rb32")
                    nc.vector.tensor_copy(rb32[:], rb[:])
                    nc.sync.dma_start(out.ap()[kk * RB : (kk + 1) * RB, :], rb32[:])

    nc.compile()
    return nc


def _get_compiled():
    global _compiled
    if _compiled is None:
        _compiled = _build()
    return _compiled


def kernel(residual, W_Q, W_K, W_V, W_O):
    nc = _get_compiled()

    resid_t = np.ascontiguousarray(residual.reshape(SEQ, D).T.astype(np.float32)).astype(NP_BF16)
    # fold the 1/sqrt(DH) score scale into W_Q
    wq2 = (W_Q.astype(np.float64) / np.sqrt(DH)).reshape(NH * DH, D).astype(np.float32)
    wqt_full = np.ascontiguousarray(wq2.T)  # [D, NH*DH]
    wkt_full = np.ascontiguousarray(W_K.reshape(NKV * DH, D).T)  # [D, NKV*DH]
    wvt_full = np.ascontiguousarray(W_V.reshape(NKV * DH, D).T)
    wo_full = W_O.reshape(NH * DH, D)  # [NH*DH, D]

    mask_np = np.triu(np.ones((P, P), dtype=np.float32)).astype(NP_BF16)  # [k, q]: q>=k
    ident_np = np.eye(P, dtype=np.float32)

    in_maps = []
    for c in range(N_CORES):
        in_maps.append(
            {
                "resid_t": resid_t,
                "wqt": np.ascontiguousarray(
                    wqt_full[:, c * NHL : (c + 1) * NHL]
                ).astype(NP_BF16),
                "wkvt": np.ascontiguousarray(
                    np.concatenate(
                        [
                            wkt_full[:, c * DH : (c + 1) * DH],
                            wvt_full[:, c * DH : (c + 1) * DH],
                        ],
                        axis=1,
                    )
                ).astype(NP_BF16),
                "wo": np.ascontiguousarray(
                    wo_full[c * NHL : (c + 1) * NHL, :]
                ).astype(NP_BF16),
                "mask": mask_np,
                "ident": ident_np,
            }
        )

    import os

    reps = int(os.environ.get("KERNEEL_REPS", os.environ.get("KERNEL_REPS", "1")))
    times = []
    for _ in range(max(1, reps)):
        res = bass_utils.run_bass_kernel_spmd(
            nc, in_maps, core_ids=list(range(N_CORES))
        )
        times.append(res.exec_time_ns)
    kernel.last_results = res
    kernel.exec_times = times

    out_full = np.empty((SEQ, D), dtype=np.float32)
    rb = QG // N_CORES  # 64 rows per chunk per core
    for c in range(N_CORES):
        shard = res.results[c]["out"]  # [512, D]
        for kk in range(N_CHUNK):
            g0 = kk * QG + rb * c
            out_full[g0 : g0 + rb] = shard[kk * rb : (kk + 1) * rb]
    return out_full.reshape(B, S, D)



# revision 2
# speedup vs baseline: 1.1056x; 1.1056x over previous
"""Distributed GQA causal attention forward on 8 TRN2 NeuronCores.

Problem shapes: residual [B=2, S=2048, D=2048]; W_Q/W_O [32, 64, 2048];
W_K/W_V [8, 64, 2048]; GQA rep=4; causal softmax attention; out [2, 2048, 2048].

Sharding (batch x kv-pair tensor parallel): core c owns batch b = c//4 and the
kv-head pair {2r, 2r+1} (r = c%4), i.e. q-heads 8r..8r+7 of that batch. Each
core computes Q/K/V projections for its 8 heads over its batch's 2048 rows,
flash-style causal attention with head pairs packed into PE row groups
(kv head A in partitions 0:63, kv head B in 64:127), and a partial output
projection in d-major layout; partials are summed with a per-chunk bf16
ReduceScatter over the 4 cores of the same batch, each rank keeping a
512-column d-slice that the host transposes/assembles.

All matmul operands are bf16 (fp32 PSUM accumulation); the scores scale
1/sqrt(64) is folded into W_Q on the host. Softmax skips max-subtraction
(logits are bounded ~|5| for this data distribution); row-sums come from a
ones-column appended to V. PSUM drains run on DVE (attention phase) / ACT
(projection phase) so the exp stream owns the ACT engine during attention.
"""

import sys

for _p in ("/opt/trn_rl_repo", "/root/.axon_site/_ro/trn_rl_repo"):
    if _p not in sys.path:
        sys.path.insert(0, _p)

import numpy as np
from concourse import bacc, mybir, tile
from concourse import bass_utils

N_CORES = 8
GROUP = 4  # ranks per ReduceScatter group (cores per batch)
B, S, D = 2, 2048, 2048
NH, NKV, DH = 32, 8, 64
P = 128
QG = 512  # q-chunk size (4 tiles of 128)
N_CHUNK = S // QG  # 4 chunks per core
N_DT = D // P  # 16 d-tiles
N_KT = S // P  # 16 key blocks
NPAIR = 4  # head pairs per core (8 q-heads)

BF16 = mybir.dt.bfloat16
F32 = mybir.dt.float32
NP_BF16 = mybir.dt.np(BF16)

_compiled = None


def _build():
    nc = bacc.Bacc("TRN2", target_bir_lowering=False, debug=False, num_devices=N_CORES)

    resid_t = nc.dram_tensor("resid_t", [D, S], BF16, kind="ExternalInput")
    wqt = nc.dram_tensor("wqt", [D, NPAIR * P], BF16, kind="ExternalInput")
    wkvt = nc.dram_tensor("wkvt", [D, 4 * DH], BF16, kind="ExternalInput")
    wo = nc.dram_tensor("wo", [NPAIR * P, D], BF16, kind="ExternalInput")
    mask = nc.dram_tensor("mask", [P, P], BF16, kind="ExternalInput")
    ident = nc.dram_tensor("ident", [P, P], F32, kind="ExternalInput")
    out = nc.dram_tensor("out", [D // GROUP, S], BF16, kind="ExternalOutput")

    rs_in = [
        nc.dram_tensor(f"rs_in{k}", [D, QG], BF16, kind="Internal")
        for k in range(N_CHUNK)
    ]
    rs_out = [
        nc.dram_tensor(f"rs_out{k}", [D // GROUP, QG], BF16, kind="Internal")
        for k in range(N_CHUNK)
    ]
    rg = [list(range(GROUP)), list(range(GROUP, 2 * GROUP))]
    COPY = mybir.ActivationFunctionType.Copy
    EXP = mybir.ActivationFunctionType.Exp

    with tile.TileContext(nc) as tc:
        with (
            tc.tile_pool(name="persist", bufs=1) as pp,
            tc.tile_pool(name="stream", bufs=3) as sp,
            tc.tile_pool(name="rstream", bufs=8) as rp,
            tc.tile_pool(name="pstream", bufs=4) as xp,
            tc.tile_pool(name="outbuf", bufs=4) as op,
        ):
            # ---- persistent SBUF tensors ----
            qT_sb = [pp.tile([P, S], BF16, name=f"qT{i}") for i in range(NPAIR)]
            kT_sb = pp.tile([P, S], BF16, name="kT")  # A in 0:64, B in 64:128
            v_sb = [
                [pp.tile([P, P], BF16, name=f"v{u}_{rt}") for rt in range(N_KT)]
                for u in range(2)
            ]
            attn_sb = [pp.tile([P, S], BF16, name=f"attn{i}") for i in range(NPAIR)]
            wqt_sb = [pp.tile([P, NPAIR * P], BF16, name=f"wqt{i}") for i in range(N_DT)]
            wkvt_sb = [pp.tile([P, 4 * DH], BF16, name=f"wkvt{i}") for i in range(N_DT)]
            wo_sb = [pp.tile([P, D], BF16, name=f"wo{i}") for i in range(NPAIR)]
            mask_sb = pp.tile([P, P], BF16, name="mask")
            ident_sb = pp.tile([P, P], F32, name="ident")
            warm_sb = pp.tile([1, 8], F32, name="warm")

            nc.sync.dma_start(mask_sb[:], mask.ap())
            nc.sync.dma_start(ident_sb[:], ident.ap())
            # warm the ACT exp table before phase B needs it
            nc.scalar.activation(warm_sb[:], ident_sb[0:1, 0:8], EXP)
            for i in range(N_DT):
                nc.sync.dma_start(wqt_sb[i][:], wqt.ap()[i * P : (i + 1) * P, :])
                nc.sync.dma_start(wkvt_sb[i][:], wkvt.ap()[i * P : (i + 1) * P, :])
            for i in range(NPAIR):
                nc.sync.dma_start(wo_sb[i][:], wo.ap()[i * P : (i + 1) * P, :])
            for u in range(2):
                for rt in range(N_KT):
                    nc.vector.memset(v_sb[u][rt][:, 0:DH], 0.0)
                    nc.vector.memset(v_sb[u][rt][:, 0:1], 1.0)

            # ---- phase A: Q / K / V projections ----
            # residual^T streamed in [128 d, 512 row] tiles (pre-transposed on
            # host); per d-tile 6 matmuls: 4 Q pair-accumulators, K (both kv
            # heads), V^T (both kv heads). Drains on ACT (idle this phase).
            with (
                tc.tile_pool(name="psA", bufs=1, space="PSUM") as psA,
                tc.tile_pool(name="psAT", bufs=2, space="PSUM") as psAT,
            ):
                for rc in range(N_CHUNK):
                    r0 = rc * QG
                    qp = [
                        psA.tile([P, QG], F32, tag=f"qp{i}", name=f"qp{i}")
                        for i in range(NPAIR)
                    ]
                    kp = psA.tile([P, QG], F32, tag="kp", name="kp")
                    vp = psA.tile([P, QG], F32, tag="vp", name="vp")
                    for dt_ in range(N_DT):
                        rt_tile = rp.tile([P, QG], BF16, tag="residT", name="residT")
                        nc.sync.dma_start(
                            rt_tile[:],
                            resid_t.ap()[dt_ * P : (dt_ + 1) * P, r0 : r0 + QG],
                        )
                        st = dict(start=(dt_ == 0), stop=(dt_ == N_DT - 1))
                        for hb in range(NPAIR):
                            nc.tensor.matmul(
                                qp[hb][:],
                                wqt_sb[dt_][:, hb * P : (hb + 1) * P],
                                rt_tile[:],
                                **st,
                            )
                        nc.tensor.matmul(kp[:], wkvt_sb[dt_][:, 0:P], rt_tile[:], **st)
                        nc.tensor.matmul(
                            vp[:], wkvt_sb[dt_][:, P : 2 * P], rt_tile[:], **st
                        )
                    for hb in range(NPAIR):
                        nc.scalar.activation(
                            qT_sb[hb][:, r0 : r0 + QG], qp[hb][:], COPY
                        )
                    nc.scalar.activation(kT_sb[:, r0 : r0 + QG], kp[:], COPY)
                    # V^T -> V via PE transpose (per 128-key tile)
                    vt_tmp = sp.tile([P, QG], F32, tag="vt_tmp", name="vt_tmp")
                    nc.scalar.activation(vt_tmp[:], vp[:], COPY)
                    for j in range(QG // P):
                        vtr = psAT.tile([P, P], F32, tag="vtr", name="vtr")
                        nc.tensor.transpose(
                            vtr[:], vt_tmp[:, j * P : (j + 1) * P], ident_sb[:]
                        )
                        kb = rc * 4 + j
                        nc.scalar.activation(
                            v_sb[0][kb][:, DH : 2 * DH], vtr[:, 0:DH], COPY
                        )
                        nc.scalar.activation(
                            v_sb[1][kb][:, DH : 2 * DH], vtr[:, DH : 2 * DH], COPY
                        )

            # ---- phases B+C interleaved per chunk ----
            # chunk g covers q rows [g*512, (g+1)*512); two passes of 2 head
            # pairs each (at = 4 PSUM accumulators per pass). Previous chunk's
            # d-major O-projection units are interleaved between key blocks;
            # drains go to DVE, mask multiplies to GpSimd-free DVE slots, row
            # normalization via DVE recip + GpSimd partition broadcast.
            with (
                tc.tile_pool(name="psS", bufs=2, space="PSUM") as psS,
                tc.tile_pool(name="psT", bufs=4, space="PSUM") as psT,
            ):

                def emit_opunit(kc, dti):
                    ops = psS.tile([P, 2, QG], F32, tag="sc", name="sc")
                    for pr in range(NPAIR):
                        nc.tensor.matmul(
                            ops[:, 0, :],
                            wo_sb[pr][:, dti * P : (dti + 1) * P],
                            attn_sb[pr][:, kc * QG : (kc + 1) * QG],
                            start=(pr == 0),
                            stop=(pr == NPAIR - 1),
                        )
                    o_sb = op.tile([P, QG], BF16, tag="o_sb", name="o_sb")
                    nc.vector.tensor_copy(o_sb[:], ops[:, 0, :])
                    return nc.sync.dma_start(
                        rs_in[kc].ap()[dti * P : (dti + 1) * P, :], o_sb[:]
                    )

                last_osb_dma = None

                def emit_rs(kc):
                    nc.gpsimd.collective_compute(
                        "ReduceScatter",
                        mybir.AluOpType.add,
                        replica_groups=rg,
                        ins=[rs_in[kc].ap().opt()],
                        outs=[rs_out[kc].ap().opt()],
                    )
                    nc.sync.dma_start(
                        out.ap()[:, kc * QG : (kc + 1) * QG], rs_out[kc].ap()
                    )

                for kk in range(N_CHUNK):
                    g = kk
                    # previous chunk's O-projection: 16 d-tiles interleaved
                    op_todo = list(range(N_DT)) if kk > 0 else []
                    for p in range(2):
                        at = [
                            psT.tile([P, QG], F32, tag="at", name="at")
                            for _ in range(4)
                        ]
                        for kb in range(4 * g + 4):
                            j = max(0, kb - 4 * g)
                            n = QG - j * P
                            qoff = g * QG + j * P
                            k0 = kb * P
                            for ii, pr in enumerate((2 * p, 2 * p + 1)):
                                sc = psS.tile([P, 2, QG], F32, tag="sc", name="sc")
                                for u in range(2):
                                    lo = u * DH
                                    nc.tensor.matmul(
                                        sc[:, u, :n],
                                        kT_sb[lo : lo + DH, k0 : k0 + P],
                                        qT_sb[pr][lo : lo + DH, qoff : qoff + n],
                                        start=True,
                                        stop=True,
                                    )
                                pt = xp.tile([P, 2, QG], BF16, tag="p_sb", name="p_sb")
                                nc.scalar.activation(pt[:, :, :n], sc[:, :, :n], EXP)
                                if kb >= 4 * g:
                                    nc.vector.tensor_tensor(
                                        pt[:, :, 0:P],
                                        pt[:, :, 0:P],
                                        mask_sb[:].unsqueeze(1).broadcast_to([P, 2, P]),
                                        mybir.AluOpType.mult,
                                    )
                                for u in range(2):
                                    nc.tensor.matmul(
                                        at[2 * ii + u][:, j * P : QG],
                                        v_sb[u][kb][:],
                                        pt[:, u, :n],
                                        start=(kb == 0),
                                        stop=(kb == 4 * g + 3),
                                    )
                            if op_todo:
                                last_osb_dma = emit_opunit(kk - 1, op_todo.pop(0))
                            if op_todo:
                                last_osb_dma = emit_opunit(kk - 1, op_todo.pop(0))
                        # normalize pass p into attn_sb
                        for ii, pr in enumerate((2 * p, 2 * p + 1)):
                            for u in range(2):
                                a = at[2 * ii + u]
                                recip = sp.tile([1, QG], F32, tag="recip", name="recip")
                                nc.vector.reciprocal_approx_fast(recip[:], a[0:1, :])
                                bc = sp.tile([DH, QG], F32, tag="bc", name="bc")
                                nc.gpsimd.partition_broadcast(bc[:], recip[:])
                                nc.vector.tensor_tensor(
                                    attn_sb[pr][
                                        u * DH : (u + 1) * DH, g * QG : (g + 1) * QG
                                    ],
                                    a[DH : 2 * DH, :],
                                    bc[:],
                                    mybir.AluOpType.mult,
                                )
                    while op_todo:
                        last_osb_dma = emit_opunit(kk - 1, op_todo.pop(0))
                    if kk > 0:
                        emit_rs(kk - 1)
                for dti in range(N_DT):
                    last_osb_dma = emit_opunit(N_CHUNK - 1, dti)
                emit_rs(N_CHUNK - 1)

    nc.compile()
    return nc


def _get_compiled():
    global _compiled
    if _compiled is None:
        _compiled = _build()
    return _compiled


def kernel(residual, W_Q, W_K, W_V, W_O):
    nc = _get_compiled()

    # fold the 1/sqrt(DH) score scale into W_Q
    wq2 = (np.asarray(W_Q, dtype=np.float64) / np.sqrt(DH)).reshape(NH * DH, D)
    wq2 = wq2.astype(np.float32)
    wk2 = np.asarray(W_K, dtype=np.float32).reshape(NKV * DH, D)
    wv2 = np.asarray(W_V, dtype=np.float32).reshape(NKV * DH, D)
    wo2 = np.asarray(W_O, dtype=np.float32).reshape(NH * DH, D)

    mask_np = np.triu(np.ones((P, P), dtype=np.float32)).astype(NP_BF16)  # [k, q]
    ident_np = np.eye(P, dtype=np.float32)

    resid_t = [
        np.ascontiguousarray(
            np.asarray(residual[b], dtype=np.float32).T
        ).astype(NP_BF16)
        for b in range(B)
    ]

    in_maps = []
    for c in range(N_CORES):
        b, r = divmod(c, GROUP)
        # head pair i: q-head a = 8r+i (kv head 2r), q-head b = 8r+4+i (kv 2r+1)
        wqt_cols = []
        wo_rows = []
        for i in range(NPAIR):
            qa = (8 * r + i) * DH
            qb = (8 * r + 4 + i) * DH
            wqt_cols.append(wq2[qa : qa + DH].T)
            wqt_cols.append(wq2[qb : qb + DH].T)
            wo_rows.append(wo2[qa : qa + DH])
            wo_rows.append(wo2[qb : qb + DH])
        wqt_full = np.ascontiguousarray(np.concatenate(wqt_cols, axis=1))  # [D, 512]
        wo_full = np.ascontiguousarray(np.concatenate(wo_rows, axis=0))  # [512, D]
        ka = 2 * r * DH
        kb_ = (2 * r + 1) * DH
        wkvt_full = np.ascontiguousarray(
            np.concatenate(
                [
                    wk2[ka : ka + DH].T,
                    wk2[kb_ : kb_ + DH].T,
                    wv2[ka : ka + DH].T,
                    wv2[kb_ : kb_ + DH].T,
                ],
                axis=1,
            )
        )  # [D, 256]
        in_maps.append(
            {
                "resid_t": resid_t[b],
                "wqt": wqt_full.astype(NP_BF16),
                "wkvt": wkvt_full.astype(NP_BF16),
                "wo": wo_full.astype(NP_BF16),
                "mask": mask_np,
                "ident": ident_np,
            }
        )

    import os

    reps = int(os.environ.get("KERNEL_REPS", "1"))
    times = []
    for _ in range(max(1, reps)):
        res = bass_utils.run_bass_kernel_spmd(
            nc, in_maps, core_ids=list(range(N_CORES))
        )
        times.append(res.exec_time_ns)
    kernel.last_results = res
    kernel.exec_times = times

    out_full = np.empty((B, S, D), dtype=np.float32)
    for c in range(N_CORES):
        b, r = divmod(c, GROUP)
        shard = np.asarray(res.results[c]["out"])  # [512 d, 2048 q] bf16
        out_full[b, :, r * (D // GROUP) : (r + 1) * (D // GROUP)] = (
            shard.astype(np.float32).T
        )
    return out_full


# revision 9
# speedup vs baseline: 1.1683x; 1.0568x over previous
"""Distributed GQA causal attention forward on 8 TRN2 NeuronCores.

Problem shapes: residual [B=2, S=2048, D=2048]; W_Q/W_O [32, 64, 2048];
W_K/W_V [8, 64, 2048]; GQA rep=4; causal softmax attention; out [2, 2048, 2048].

Sharding (batch x kv-pair tensor parallel): core c owns batch b = c//4 and the
kv-head pair {2r, 2r+1} (r = c%4), i.e. q-heads 8r..8r+7 of that batch. Each
core computes Q/K/V projections for its 8 heads over its batch's 2048 rows,
flash-style causal attention with head pairs packed into PE row groups
(kv head A in partitions 0:63, kv head B in 64:127), and a partial output
projection in d-major layout; partials are summed with a per-chunk bf16
ReduceScatter over the 4 cores of the same batch, each rank keeping a
512-column d-slice that the host transposes/assembles.

All matmul operands are bf16 (fp32 PSUM accumulation); the scores scale
1/sqrt(64) is folded into W_Q on the host. Softmax skips max-subtraction
(logits are bounded ~|5| for this data distribution); row-sums come from a
ones-column appended to V. PSUM drains run on DVE (attention phase) / ACT
(projection phase) so the exp stream owns the ACT engine during attention.
"""

import sys

for _p in ("/opt/trn_rl_repo", "/root/.axon_site/_ro/trn_rl_repo"):
    if _p not in sys.path:
        sys.path.insert(0, _p)

import numpy as np
from concourse import bacc, mybir, tile
from concourse import bass_utils

N_CORES = 8
GROUP = 4  # ranks per ReduceScatter group (cores per batch)
B, S, D = 2, 2048, 2048
NH, NKV, DH = 32, 8, 64
P = 128
QG = 512  # q-chunk size (4 tiles of 128)
N_CHUNK = S // QG  # 4 chunks per core
N_DT = D // P  # 16 d-tiles
N_KT = S // P  # 16 key blocks
NPAIR = 4  # head pairs per core (8 q-heads)

BF16 = mybir.dt.bfloat16
F32 = mybir.dt.float32
NP_BF16 = mybir.dt.np(BF16)

_compiled = None


def _build():
    nc = bacc.Bacc("TRN2", target_bir_lowering=False, debug=False, num_devices=N_CORES)

    resid_t = nc.dram_tensor("resid_t", [D, S], BF16, kind="ExternalInput")
    wqt = nc.dram_tensor("wqt", [D, NPAIR * P], BF16, kind="ExternalInput")
    wkvt = nc.dram_tensor("wkvt", [D, 4 * DH], BF16, kind="ExternalInput")
    wo = nc.dram_tensor("wo", [NPAIR * P, D], BF16, kind="ExternalInput")
    mask = nc.dram_tensor("mask", [P, P], BF16, kind="ExternalInput")
    ident = nc.dram_tensor("ident", [P, P], F32, kind="ExternalInput")
    out = nc.dram_tensor("out", [D // GROUP, S - QG], BF16, kind="ExternalOutput")
    out_last = nc.dram_tensor("out_last", [D // GROUP, QG], BF16, kind="ExternalOutput")

    rs_in = [
        nc.dram_tensor(f"rs_in{k}", [D, QG], BF16, kind="Internal")
        for k in range(N_CHUNK)
    ]
    rs_out = [
        nc.dram_tensor(f"rs_out{k}", [D // GROUP, QG], BF16, kind="Internal")
        for k in range(N_CHUNK - 1)
    ]
    # last chunk's RS is split into 4 sub-collectives (4 d-tiles each) so the
    # tail after the final O-projection is one small RS, not a full-chunk one
    rs_out_last = [
        nc.dram_tensor(f"rs_outL{k}", [P, QG], BF16, kind="Internal")
        for k in range(4)
    ]
    rg = [list(range(GROUP)), list(range(GROUP, 2 * GROUP))]
    COPY = mybir.ActivationFunctionType.Copy
    EXP = mybir.ActivationFunctionType.Exp

    with tile.TileContext(nc) as tc:
        with (
            tc.tile_pool(name="persist", bufs=1) as pp,
            tc.tile_pool(name="stream", bufs=3) as sp,
            tc.tile_pool(name="rstream", bufs=8) as rp,
            tc.tile_pool(name="pstream", bufs=4) as xp,
            tc.tile_pool(name="outbuf", bufs=4) as op,
        ):
            # ---- persistent SBUF tensors ----
            qT_sb = [pp.tile([P, S], BF16, name=f"qT{i}") for i in range(NPAIR)]
            kT_sb = pp.tile([P, S], BF16, name="kT")  # A in 0:64, B in 64:128
            v_sb = [
                [pp.tile([P, P], BF16, name=f"v{u}_{rt}") for rt in range(N_KT)]
                for u in range(2)
            ]
            attn_sb = [pp.tile([P, S], BF16, name=f"attn{i}") for i in range(NPAIR)]
            wqt_sb = [pp.tile([P, NPAIR * P], BF16, name=f"wqt{i}") for i in range(N_DT)]
            wkvt_sb = [pp.tile([P, 4 * DH], BF16, name=f"wkvt{i}") for i in range(N_DT)]
            wo_sb = [pp.tile([P, D], BF16, name=f"wo{i}") for i in range(NPAIR)]
            mask_sb = pp.tile([P, P], BF16, name="mask")
            ident_sb = pp.tile([P, P], F32, name="ident")
            warm_sb = pp.tile([1, 8], F32, name="warm")

            nc.sync.dma_start(mask_sb[:], mask.ap())
            nc.sync.dma_start(ident_sb[:], ident.ap())
            # warm the ACT exp table before phase B needs it
            nc.scalar.activation(warm_sb[:], ident_sb[0:1, 0:8], EXP)
            # only d-tile 0's weights up front; the rest stream just-in-time
            # during chunk 0's d-loop so the first matmul isn't DMA-gated
            nc.sync.dma_start(wqt_sb[0][:], wqt.ap()[0:P, :])
            nc.sync.dma_start(wkvt_sb[0][:], wkvt.ap()[0:P, :])
            for u in range(2):
                for rt in range(N_KT):
                    nc.vector.memset(v_sb[u][rt][:, 0:DH], 0.0)
                    nc.vector.memset(v_sb[u][rt][:, 0:1], 1.0)

            # ---- phase A: Q / K / V projections ----
            # residual^T streamed in [128 d, 512 row] tiles (pre-transposed on
            # host); per d-tile 6 matmuls: 4 Q pair-accumulators, K (both kv
            # heads), V^T (both kv heads). Drains on ACT (idle this phase).
            with (
                tc.tile_pool(name="psA", bufs=1, space="PSUM") as psA,
                tc.tile_pool(name="psAT", bufs=2, space="PSUM") as psAT,
            ):
                for rc in range(N_CHUNK):
                    r0 = rc * QG
                    qp = [
                        psA.tile([P, QG], F32, tag=f"qp{i}", name=f"qp{i}")
                        for i in range(NPAIR)
                    ]
                    kp = psA.tile([P, QG], F32, tag="kp", name="kp")
                    vp = psA.tile([P, QG], F32, tag="vp", name="vp")
                    for dt_ in range(N_DT):
                        rt_tile = rp.tile([P, QG], BF16, tag="residT", name="residT")
                        nc.sync.dma_start(
                            rt_tile[:],
                            resid_t.ap()[dt_ * P : (dt_ + 1) * P, r0 : r0 + QG],
                        )
                        if rc == 0 and dt_ < N_DT - 1:
                            # JIT weight prefetch, one d-tile ahead
                            nc.sync.dma_start(
                                wqt_sb[dt_ + 1][:],
                                wqt.ap()[(dt_ + 1) * P : (dt_ + 2) * P, :],
                            )
                            nc.sync.dma_start(
                                wkvt_sb[dt_ + 1][:],
                                wkvt.ap()[(dt_ + 1) * P : (dt_ + 2) * P, :],
                            )
                        if rc == 1 and dt_ < NPAIR:
                            # W_O streams during chunk 1, needed first ~80us in
                            nc.sync.dma_start(
                                wo_sb[dt_][:], wo.ap()[dt_ * P : (dt_ + 1) * P, :]
                            )
                        st = dict(start=(dt_ == 0), stop=(dt_ == N_DT - 1))
                        for hb in range(NPAIR):
                            nc.tensor.matmul(
                                qp[hb][:],
                                wqt_sb[dt_][:, hb * P : (hb + 1) * P],
                                rt_tile[:],
                                **st,
                            )
                        nc.tensor.matmul(kp[:], wkvt_sb[dt_][:, 0:P], rt_tile[:], **st)
                        nc.tensor.matmul(
                            vp[:], wkvt_sb[dt_][:, P : 2 * P], rt_tile[:], **st
                        )
                    for hb in range(NPAIR):
                        nc.scalar.activation(
                            qT_sb[hb][:, r0 : r0 + QG], qp[hb][:], COPY
                        )
                    nc.scalar.activation(kT_sb[:, r0 : r0 + QG], kp[:], COPY)
                    # V^T -> V via PE transpose (per 128-key tile)
                    vt_tmp = sp.tile([P, QG], F32, tag="vt_tmp", name="vt_tmp")
                    nc.scalar.activation(vt_tmp[:], vp[:], COPY)
                    for j in range(QG // P):
                        vtr = psAT.tile([P, P], F32, tag="vtr", name="vtr")
                        nc.tensor.transpose(
                            vtr[:], vt_tmp[:, j * P : (j + 1) * P], ident_sb[:]
                        )
                        kb = rc * 4 + j
                        nc.scalar.activation(
                            v_sb[0][kb][:, DH : 2 * DH], vtr[:, 0:DH], COPY
                        )
                        nc.scalar.activation(
                            v_sb[1][kb][:, DH : 2 * DH], vtr[:, DH : 2 * DH], COPY
                        )

            # ---- phases B+C interleaved per chunk ----
            # chunk g covers q rows [g*512, (g+1)*512); two passes of 2 head
            # pairs each (at = 4 PSUM accumulators per pass). Previous chunk's
            # d-major O-projection units are interleaved between key blocks;
            # drains go to DVE, mask multiplies to GpSimd-free DVE slots, row
            # normalization via DVE recip + GpSimd partition broadcast.
            with (
                tc.tile_pool(name="psS", bufs=2, space="PSUM") as psS,
                tc.tile_pool(name="psT", bufs=4, space="PSUM") as psT,
            ):

                def emit_opunit(kc, dti):
                    ops = psS.tile([P, 2, QG], F32, tag="sc", name="sc")
                    for pr in range(NPAIR):
                        nc.tensor.matmul(
                            ops[:, 0, :],
                            wo_sb[pr][:, dti * P : (dti + 1) * P],
                            attn_sb[pr][:, kc * QG : (kc + 1) * QG],
                            start=(pr == 0),
                            stop=(pr == NPAIR - 1),
                        )
                    o_sb = op.tile([P, QG], BF16, tag="o_sb", name="o_sb")
                    nc.vector.tensor_copy(o_sb[:], ops[:, 0, :])
                    return nc.sync.dma_start(
                        rs_in[kc].ap()[dti * P : (dti + 1) * P, :], o_sb[:]
                    )

                last_osb_dma = None

                def emit_rs(kc):
                    nc.gpsimd.collective_compute(
                        "ReduceScatter",
                        mybir.AluOpType.add,
                        replica_groups=rg,
                        ins=[rs_in[kc].ap().opt()],
                        outs=[rs_out[kc].ap().opt()],
                    )

                for kk in range(N_CHUNK):
                    g = kk
                    # previous chunk's O-projection: 16 d-tiles interleaved
                    op_todo = list(range(N_DT)) if kk > 0 else []
                    for p in range(2):
                        at = [
                            psT.tile([P, QG], F32, tag="at", name="at")
                            for _ in range(4)
                        ]
                        for kb in range(4 * g + 4):
                            j = max(0, kb - 4 * g)
                            n = QG - j * P
                            qoff = g * QG + j * P
                            k0 = kb * P
                            for ii, pr in enumerate((2 * p, 2 * p + 1)):
                                sc = psS.tile([P, 2, QG], F32, tag="sc", name="sc")
                                for u in range(2):
                                    lo = u * DH
                                    nc.tensor.matmul(
                                        sc[:, u, :n],
                                        kT_sb[lo : lo + DH, k0 : k0 + P],
                                        qT_sb[pr][lo : lo + DH, qoff : qoff + n],
                                        start=True,
                                        stop=True,
                                    )
                                pt = xp.tile([P, 2, QG], BF16, tag="p_sb", name="p_sb")
                                nc.scalar.activation(pt[:, :, :n], sc[:, :, :n], EXP)
                                if kb >= 4 * g:
                                    nc.vector.tensor_tensor(
                                        pt[:, :, 0:P],
                                        pt[:, :, 0:P],
                                        mask_sb[:].unsqueeze(1).broadcast_to([P, 2, P]),
                                        mybir.AluOpType.mult,
                                    )
                                for u in range(2):
                                    nc.tensor.matmul(
                                        at[2 * ii + u][:, j * P : QG],
                                        v_sb[u][kb][:],
                                        pt[:, u, :n],
                                        start=(kb == 0),
                                        stop=(kb == 4 * g + 3),
                                    )
                            if op_todo:
                                last_osb_dma = emit_opunit(kk - 1, op_todo.pop(0))
                            if op_todo:
                                last_osb_dma = emit_opunit(kk - 1, op_todo.pop(0))
                        # normalize pass p into attn_sb
                        for ii, pr in enumerate((2 * p, 2 * p + 1)):
                            for u in range(2):
                                a = at[2 * ii + u]
                                recip = sp.tile([1, QG], F32, tag="recip", name="recip")
                                nc.vector.reciprocal_approx_fast(recip[:], a[0:1, :])
                                bc = sp.tile([DH, QG], F32, tag="bc", name="bc")
                                nc.gpsimd.partition_broadcast(bc[:], recip[:])
                                nc.vector.tensor_tensor(
                                    attn_sb[pr][
                                        u * DH : (u + 1) * DH, g * QG : (g + 1) * QG
                                    ],
                                    a[DH : 2 * DH, :],
                                    bc[:],
                                    mybir.AluOpType.mult,
                                )
                    while op_todo:
                        last_osb_dma = emit_opunit(kk - 1, op_todo.pop(0))
                    if kk > 0:
                        emit_rs(kk - 1)
                # last chunk's O-projection, RS split 4 ways so each sub-RS
                # starts as soon as its 4 d-tiles are drained
                kl = N_CHUNK - 1
                for gi in range(4):
                    for dti in range(4 * gi, 4 * gi + 4):
                        last_osb_dma = emit_opunit(kl, dti)
                    nc.gpsimd.collective_compute(
                        "ReduceScatter",
                        mybir.AluOpType.add,
                        replica_groups=rg,
                        ins=[rs_in[kl].ap()[4 * gi * P : (4 * gi + 4) * P, :].opt()],
                        outs=[rs_out_last[gi].ap().opt()],
                    )
                # readback DMAs at the very end of the sync queue: their RS
                # waits can't head-of-line block any compute-feeding DMA
                for kc in range(N_CHUNK - 1):
                    nc.sync.dma_start(
                        out.ap()[:, kc * QG : (kc + 1) * QG], rs_out[kc].ap()
                    )
                for gi in range(4):
                    nc.sync.dma_start(
                        out_last.ap()[gi * P : (gi + 1) * P, :], rs_out_last[gi].ap()
                    )

    nc.compile()
    return nc


def _get_compiled():
    global _compiled
    if _compiled is None:
        _compiled = _build()
    return _compiled


def kernel(residual, W_Q, W_K, W_V, W_O):
    nc = _get_compiled()

    # fold the 1/sqrt(DH) score scale into W_Q
    wq2 = (np.asarray(W_Q, dtype=np.float64) / np.sqrt(DH)).reshape(NH * DH, D)
    wq2 = wq2.astype(np.float32)
    wk2 = np.asarray(W_K, dtype=np.float32).reshape(NKV * DH, D)
    wv2 = np.asarray(W_V, dtype=np.float32).reshape(NKV * DH, D)
    wo2 = np.asarray(W_O, dtype=np.float32).reshape(NH * DH, D)

    mask_np = np.triu(np.ones((P, P), dtype=np.float32)).astype(NP_BF16)  # [k, q]
    ident_np = np.eye(P, dtype=np.float32)

    resid_t = [
        np.ascontiguousarray(
            np.asarray(residual[b], dtype=np.float32).T
        ).astype(NP_BF16)
        for b in range(B)
    ]

    in_maps = []
    for c in range(N_CORES):
        b, r = divmod(c, GROUP)
        # head pair i: q-head a = 8r+i (kv head 2r), q-head b = 8r+4+i (kv 2r+1)
        wqt_cols = []
        wo_rows = []
        for i in range(NPAIR):
            qa = (8 * r + i) * DH
            qb = (8 * r + 4 + i) * DH
            wqt_cols.append(wq2[qa : qa + DH].T)
            wqt_cols.append(wq2[qb : qb + DH].T)
            wo_rows.append(wo2[qa : qa + DH])
            wo_rows.append(wo2[qb : qb + DH])
        wqt_full = np.ascontiguousarray(np.concatenate(wqt_cols, axis=1))  # [D, 512]
        wo_full = np.ascontiguousarray(np.concatenate(wo_rows, axis=0))  # [512, D]
        ka = 2 * r * DH
        kb_ = (2 * r + 1) * DH
        wkvt_full = np.ascontiguousarray(
            np.concatenate(
                [
                    wk2[ka : ka + DH].T,
                    wk2[kb_ : kb_ + DH].T,
                    wv2[ka : ka + DH].T,
                    wv2[kb_ : kb_ + DH].T,
                ],
                axis=1,
            )
        )  # [D, 256]
        in_maps.append(
            {
                "resid_t": resid_t[b],
                "wqt": wqt_full.astype(NP_BF16),
                "wkvt": wkvt_full.astype(NP_BF16),
                "wo": wo_full.astype(NP_BF16),
                "mask": mask_np,
                "ident": ident_np,
            }
        )

    import os

    reps = int(os.environ.get("KERNEL_REPS", "1"))
    times = []
    for _ in range(max(1, reps)):
        res = bass_utils.run_bass_kernel_spmd(
            nc, in_maps, core_ids=list(range(N_CORES))
        )
        times.append(res.exec_time_ns)
    kernel.last_results = res
    kernel.exec_times = times

    out_full = np.empty((B, S, D), dtype=np.float32)
    for c in range(N_CORES):
        b, r = divmod(c, GROUP)
        shard = np.asarray(res.results[c]["out"])  # [512 d, 1536 q] bf16
        out_full[b, : S - QG, r * (D // GROUP) : (r + 1) * (D // GROUP)] = (
            shard.astype(np.float32).T
        )
        last = np.asarray(res.results[c]["out_last"])  # [4*128 d, 512 q] bf16
        for gi in range(4):
            d0 = gi * QG + r * P
            out_full[b, S - QG :, d0 : d0 + P] = (
                last[gi * P : (gi + 1) * P, :].astype(np.float32).T
            )
    return out_full
